# revision 25
# baseline (speedup 1.0000x reference)
"""Causal single-head attention (B=4, S=2048, D=768) on 8 TRN2 NeuronCores.

Sharding: core (b, h) = batch b, query-interleave h. Each core computes the
attention output for query tiles {2k+h : k=0..7} (128 rows each) of one
batch. Keys are fed ROTATED by 128*h so every core sees the identical score
structure: query tile k sits at rotated row 256k and attends rotated key
tiles 0..2k (tile 2k triangular) plus the wrap tile 15, which holds the
original first 128 keys for h=1 and is killed via the exp bias for h=0.
This balances causal work exactly across the core pair.

QK merge: scores = x (W_q^T W_k) x^T, with M = W_q^T W_k computed on the
host. The kernel computes TT = M^T xq^T (one projection instead of Q and K)
and uses the resident x tiles directly as the score stationaries, removing
the K projection from the device entirely.

All matmul inputs are bf16 (1 cycle/row on the PE, same as f32r, but half
the DMA bytes); accumulation stays f32 in PSUM. Softmax denominators come
from two ones-columns appended to V (cols 768..769), divided out on the way
to the output.
"""

import os
import numpy as np
import ml_dtypes

import concourse.bass as bass
import concourse.mybir as mybir
import concourse.tile as tile
from concourse import bacc
from concourse.bass_utils import run_bass_kernel_spmd

B, S, D = 4, 2048, 768
P = 128
ND = D // P          # 6 contraction tiles
NQT = 8              # query tiles per core (128 rows each)
H = NQT * P          # 1024 query rows per core
NK = S // P          # 16 key tiles
SCALE = 1.0 / float(np.sqrt(D))
NEG = -10000.0
F32 = mybir.dt.float32
BF16 = mybir.dt.bfloat16
BF = ml_dtypes.bfloat16

_cached = {}
last_results = None


def _k0(j):
    # first query tile whose score group includes key tile j (j < 15)
    return (j + 1) // 2


def _scores_phase(nc, tc, fb, xk, xcol, tts, ptp, dgp, pss):
    pts = {}
    for j in [15] + list(range(15)):
        k0 = 0 if j == 15 else _k0(j)
        ncol = (NQT - k0) * P
        pt = ptp.tile([P, H], BF16)
        pts[j] = pt
        for qoff in range(0, ncol, 512):
            qw = min(512, ncol - qoff)
            st = pss.tile([P, 512], F32, tag="st")
            for dp in range(ND):
                nc.tensor.matmul(
                    st[:, :qw],
                    xk[:, xcol(dp, j):xcol(dp, j) + P],
                    tts[dp][:, k0 * P + qoff:k0 * P + qoff + qw],
                    start=(dp == 0), stop=(dp == ND - 1),
                )
            if j == 15:
                nc.scalar.activation(
                    pt[:, qoff:qoff + qw], st[:, :qw],
                    mybir.ActivationFunctionType.Exp,
                    bias=fb[:, 0:1], scale=SCALE,
                )
            elif qoff == 0 and j % 2 == 0:
                # even j: leading tile is the diagonal triangle; odd j's
                # leading tile is already strictly below the diagonal
                mw = min(256, ncol)
                dg = dgp.tile([P, 256], F32, tag="dg")
                nc.vector.tensor_copy(dg[:, :mw], st[:, 0:mw])
                # keep where 256*k0 + 256*kk + f - (128*j + p) >= 0
                nc.gpsimd.affine_select(
                    out=dg[:, :mw], in_=dg[:, :mw],
                    compare_op=mybir.AluOpType.is_ge,
                    fill=NEG, base=256 * k0 - P * j,
                    pattern=([[256, 2], [1, P]] if mw == 256 else [[1, P]]),
                    channel_multiplier=-1,
                )
                nc.scalar.activation(
                    pt[:, 0:mw], dg[:, :mw],
                    mybir.ActivationFunctionType.Exp,
                    bias=0.0, scale=SCALE,
                )
                if qw > mw:
                    nc.scalar.activation(
                        pt[:, mw:qw], st[:, mw:qw],
                        mybir.ActivationFunctionType.Exp,
                        bias=0.0, scale=SCALE,
                    )
            else:
                nc.scalar.activation(
                    pt[:, qoff:qoff + qw], st[:, :qw],
                    mybir.ActivationFunctionType.Exp,
                    bias=0.0, scale=SCALE,
                )
    return pts


def _build_nc():
    nc = bacc.Bacc("TRN2", target_bir_lowering=False)

    # all inputs host-packed to [128, W] with the 6 d-blocks concatenated
    # along columns -> one DMA descriptor per partition (KB-scale elements)
    m_d = nc.dram_tensor("m", [P, ND * D], BF16, kind="ExternalInput")
    xqA_d = nc.dram_tensor("xqA", [P, ND * 512], BF16, kind="ExternalInput")
    xqB_d = nc.dram_tensor("xqB", [P, ND * 512], BF16, kind="ExternalInput")
    # x keys packed chunk-major: col = 3072*c + 512*d + (j%4)*128, c = j//4
    xk_d = nc.dram_tensor("xk", [P, ND * S], BF16, kind="ExternalInput")
    # x keys again in [j, d] row layout + two ones columns, per-tile blocks
    x2_d = nc.dram_tensor("x2", [P, NK * (D + 2)], BF16, kind="ExternalInput")
    wvT_d = nc.dram_tensor("wvT", [P, ND * D], BF16, kind="ExternalInput")
    fb_d = nc.dram_tensor("fbias", [P, 1], F32, kind="ExternalInput")
    out_d = nc.dram_tensor("out", [H, D], F32, kind="ExternalOutput")

    with tile.TileContext(nc) as tc:
        with (
            tc.tile_pool(name="cst", bufs=1) as cst,
            tc.tile_pool(name="xp", bufs=1) as xp,
            tc.tile_pool(name="x2p", bufs=1) as x2p,
            tc.tile_pool(name="wvp", bufs=1) as wvp,
            tc.tile_pool(name="ttp", bufs=ND) as ttp,
            tc.tile_pool(name="ptp", bufs=NK) as ptp,
            tc.tile_pool(name="dgp", bufs=2) as dgp,
            tc.tile_pool(name="sgp", bufs=2) as sgp,
            tc.tile_pool(name="pxp", bufs=3) as pxp,
            tc.tile_pool(name="pxtp", bufs=NQT) as pxtp,
            tc.tile_pool(name="op", bufs=2) as op,
        ):
            fb = cst.tile([P, 1], F32)
            nc.sync.dma_start(out=fb[:], in_=fb_d[:, :])

            tts = []
            xk = xp.tile([P, ND * S], BF16)
            x2 = x2p.tile([P, NK * (D + 2)], BF16)
            wvsb = wvp.tile([P, ND * D], BF16)
            # ---- TT projection (m/xq pools scoped so their SBUF+PSUM free) ----
            with (
                tc.tile_pool(name="mp", bufs=1) as mp,
                tc.tile_pool(name="xqp", bufs=2) as xqp,
                tc.tile_pool(name="psj", bufs=4, space="PSUM") as psj,
            ):
                # two HW queues in parallel for the TT critical path:
                # scalar queue streams m in et-chunks (group et needs only
                # chunk et) then xqB, x2, wv; sync queue delivers xqA then
                # the key chunks in scores-consumption order (c3 first).
                msb = mp.tile([P, ND * D], BF16)
                for et in range(ND):
                    nc.scalar.dma_start(
                        out=msb[:, D * et:D * (et + 1)],
                        in_=m_d[:, D * et:D * (et + 1)])
                xqB = xqp.tile([P, ND * 512], BF16)
                nc.scalar.dma_start(out=xqB[:], in_=xqB_d[:, :])
                xqA = xqp.tile([P, ND * 512], BF16)
                nc.sync.dma_start(out=xqA[:], in_=xqA_d[:, :])
                for c in (3, 0, 1, 2):
                    nc.sync.dma_start(
                        out=xk[:, 3072 * c:3072 * (c + 1)],
                        in_=xk_d[:, 3072 * c:3072 * (c + 1)])
                for half in range(2):
                    w = NK * (D + 2) // 2
                    nc.scalar.dma_start(
                        out=x2[:, w * half:w * (half + 1)],
                        in_=x2_d[:, w * half:w * (half + 1)])
                nc.scalar.dma_start(out=wvsb[:], in_=wvT_d[:, :])

                def xcol(d, j):
                    # column of key tile j's d-block in the chunk-major layout
                    return 3072 * (j // 4) + 512 * d + 128 * (j % 4)

                # TT[d', i] = sum_d M[d, d'] xq^T[d, i]  (qc outer: the first
                # six groups only need m + xqA)
                for et in range(ND):
                    tt = ttp.tile([P, H], BF16)
                    tts.append(tt)
                for qi, xq in enumerate((xqA, xqB)):
                    for et in range(ND):
                        acc = psj.tile([P, 512], F32, tag="ps")
                        for d in range(ND):
                            nc.tensor.matmul(
                                acc[:],
                                msb[:, D * et + P * d:D * et + P * (d + 1)],
                                xq[:, 512 * d:512 * (d + 1)],
                                start=(d == 0), stop=(d == ND - 1),
                            )
                        nc.vector.tensor_copy(tts[et][:, 512 * qi:512 * (qi + 1)], acc[:])

            # ---- scores + exp -> PT tiles (own PSUM scope) ----
            with tc.tile_pool(name="pss", bufs=3, space="PSUM") as pss:
                pts = _scores_phase(nc, tc, fb, xk, xcol, tts, ptp, dgp, pss)

            # ---- Px = P @ [x | 1 1] per query tile k, then out = Pxn @ wv^T.
            # The ones columns give the softmax denominator in px[:, 768];
            # the divide lands on the bf16 Pxn copy; the d<->i transpose for
            # the final contraction runs on the DMA xbar, not the PE. fin(k)
            # is scheduled after px(k-1) so the divide+transpose latency of
            # px(k) hides under the px(k-1) matmul chain.
            with (
                tc.tile_pool(name="ppx", bufs=2, space="PSUM") as ppx,
                tc.tile_pool(name="pfin", bufs=2, space="PSUM") as pfin,
            ):
                def px_chain(k):
                    px = ppx.tile([P, D + 2], F32, tag="px")
                    js = [15] + list(range(2 * k + 1))
                    for idx, j in enumerate(js):
                        k0 = 0 if j == 15 else _k0(j)
                        koff = (k - k0) * P
                        for e0, ew in ((0, 512), (512, D + 2 - 512)):
                            nc.tensor.matmul(
                                px[:, e0:e0 + ew],
                                pts[j][:, koff:koff + P],
                                x2[:, (D + 2) * j + e0:(D + 2) * j + e0 + ew],
                                start=(idx == 0), stop=(idx == len(js) - 1),
                            )
                    rcp = sgp.tile([P, 1], F32, tag="rcp")
                    nc.vector.reciprocal(rcp[:], px[:, D:D + 1])
                    pxn = pxp.tile([P, D], BF16, tag="pxn")
                    nc.vector.tensor_scalar_mul(pxn[:], px[:, :D], rcp[:])
                    pxt = pxtp.tile([P, ND, P], BF16, tag="pxt")
                    nc.sync.dma_start_transpose(out=pxt[:], in_=pxn[:])
                    return pxt

                def fin_chain(k, pxt):
                    fin = pfin.tile([P, D], F32, tag="fin")
                    for di in range(ND):
                        for e0, ew in ((0, 512), (512, 256)):
                            nc.tensor.matmul(
                                fin[:, e0:e0 + ew],
                                pxt[:, di, :],
                                wvsb[:, D * di + e0:D * di + e0 + ew],
                                start=(di == 0), stop=(di == ND - 1),
                            )
                    o = op.tile([P, D], F32, tag="o")
                    nc.vector.tensor_copy(o[:], fin[:])
                    nc.sync.dma_start(out=out_d[k * P:(k + 1) * P, :], in_=o[:])

                # all px chains first (divides + xbar transposes trail on
                # Vector/DMA), then all fin chains - by fin time every pxt
                # is ready, so the PE never waits on the transpose latency
                pxts = {k: px_chain(k) for k in range(NQT - 1, -1, -1)}
                for k in range(NQT - 1, -1, -1):
                    fin_chain(k, pxts.pop(k))

    nc.compile()
    return nc


def _get_nc():
    if "nc" not in _cached:
        _cached["nc"] = _build_nc()
    return _cached["nc"]


def kernel(x, w_q, w_k, w_v):
    global last_results
    x = np.ascontiguousarray(np.asarray(x, dtype=np.float32))
    w_q = np.asarray(w_q, dtype=np.float32)
    w_k = np.asarray(w_k, dtype=np.float32)
    w_v = np.asarray(w_v, dtype=np.float32)

    def pack_w(w):
        # [768, 768] -> [128, 6*768] with d-blocks along columns
        return np.ascontiguousarray(
            w.reshape(ND, P, D).transpose(1, 0, 2).reshape(P, ND * D)).astype(BF)

    # m packed et-major: col = 768*et + 128*d + c  ->  M[128d+p, 128et+c]
    m = np.ascontiguousarray(
        (w_q.T @ w_k).reshape(ND, P, ND, P).transpose(1, 2, 0, 3).reshape(P, ND * D)
    ).astype(BF)
    wvT = pack_w(np.ascontiguousarray(w_v.T))

    nc = _get_nc()
    in_maps = []
    for core in range(8):
        b, h = core // 2, core % 2
        r = P * h
        rot = np.concatenate([x[b, r:], x[b, :r]], axis=0)
        xT = np.ascontiguousarray(rot.T)                      # [768, 2048]
        xk = np.ascontiguousarray(
            xT.reshape(ND, P, 4, 512).transpose(1, 2, 0, 3).reshape(P, ND * S)
        ).astype(BF)
        xqT = np.ascontiguousarray(
            x[b].reshape(NK, P, D)[h::2].reshape(H, D).T)     # [768, 1024]
        xqA = np.ascontiguousarray(
            xqT[:, 0:512].reshape(ND, P, 512).transpose(1, 0, 2).reshape(P, ND * 512)
        ).astype(BF)
        xqB = np.ascontiguousarray(
            xqT[:, 512:H].reshape(ND, P, 512).transpose(1, 0, 2).reshape(P, ND * 512)
        ).astype(BF)
        x2 = np.ascontiguousarray(
            np.concatenate([rot, np.ones((S, 2), np.float32)], axis=1)
            .reshape(NK, P, D + 2).transpose(1, 0, 2).reshape(P, NK * (D + 2))
        ).astype(BF)
        in_maps.append({
            "m": m,
            "xqA": xqA,
            "xqB": xqB,
            "xk": xk,
            "x2": x2,
            "wvT": wvT,
            "fbias": np.full((P, 1), 0.0 if h == 1 else NEG, np.float32),
        })

    trace = bool(int(os.environ.get("KERNEL_TRACE", "0")))
    res = run_bass_kernel_spmd(nc, in_maps, core_ids=list(range(8)), trace=trace)
    last_results = res

    out = np.empty((B, S, D), np.float32)
    for core in range(8):
        b, h = core // 2, core % 2
        o = res.results[core]["out"]
        out[b].reshape(NK, P, D)[h::2] = o.reshape(NQT, P, D)
    return out


# revision 28
# speedup vs baseline: 1.0762x; 1.0762x over previous
"""Causal single-head attention (B=4, S=2048, D=768) on 8 TRN2 NeuronCores.

Sharding: core (b, h) = batch b, query-interleave h. Each core computes the
attention output for query tiles {2k+h : k=0..7} (128 rows each) of one
batch. Keys are fed ROTATED by 128*h so every core sees the identical score
structure: query tile k sits at rotated row 256k and attends rotated key
tiles 0..2k (tile 2k triangular) plus the wrap tile 15, which holds the
original first 128 keys for h=1 and is killed via the exp bias for h=0.
This balances causal work exactly across the core pair.

QK merge: scores = x (W_q^T W_k) x^T, with M = W_q^T W_k computed on the
host. The kernel computes TT = M^T xq^T (one projection instead of Q and K)
and uses the resident x tiles directly as the score stationaries, removing
the K projection from the device entirely.

All matmul inputs are bf16 (1 cycle/row on the PE, same as f32r, but half
the DMA bytes); accumulation stays f32 in PSUM. Softmax denominators come
from two ones-columns appended to V (cols 768..769), divided out on the way
to the output.
"""

import os
import numpy as np
import ml_dtypes

import concourse.bass as bass
import concourse.mybir as mybir
import concourse.tile as tile
from concourse import bacc
from concourse.bass_utils import run_bass_kernel_spmd

B, S, D = 4, 2048, 768
P = 128
ND = D // P          # 6 contraction tiles
NQT = 8              # query tiles per core (128 rows each)
H = NQT * P          # 1024 query rows per core
NK = S // P          # 16 key tiles
SCALE = 1.0 / float(np.sqrt(D))
NEG = -10000.0
F32 = mybir.dt.float32
BF16 = mybir.dt.bfloat16
BF = ml_dtypes.bfloat16

_cached = {}
last_results = None


def _k0(j):
    # first query tile whose score group includes key tile j (j < 15)
    return (j + 1) // 2


def _scores_phase(nc, tc, fb, xk, xcol, tts, ptp, dgp, pss):
    pts = {}
    for j in [15] + list(range(15)):
        k0 = 0 if j == 15 else _k0(j)
        ncol = (NQT - k0) * P
        pt = ptp.tile([P, H], BF16)
        pts[j] = pt
        for qoff in range(0, ncol, 512):
            qw = min(512, ncol - qoff)
            st = pss.tile([P, 512], F32, tag="st")
            for dp in range(ND):
                nc.tensor.matmul(
                    st[:, :qw],
                    xk[:, xcol(dp, j):xcol(dp, j) + P],
                    tts[dp][:, k0 * P + qoff:k0 * P + qoff + qw],
                    start=(dp == 0), stop=(dp == ND - 1),
                )
            if j == 15:
                nc.scalar.activation(
                    pt[:, qoff:qoff + qw], st[:, :qw],
                    mybir.ActivationFunctionType.Exp,
                    bias=fb[:, 0:1], scale=SCALE,
                )
            elif qoff == 0 and j % 2 == 0:
                # even j: leading tile is the diagonal triangle; odd j's
                # leading tile is already strictly below the diagonal
                mw = min(256, ncol)
                dg = dgp.tile([P, 256], F32, tag="dg")
                nc.vector.tensor_copy(dg[:, :mw], st[:, 0:mw])
                # keep where 256*k0 + 256*kk + f - (128*j + p) >= 0
                nc.gpsimd.affine_select(
                    out=dg[:, :mw], in_=dg[:, :mw],
                    compare_op=mybir.AluOpType.is_ge,
                    fill=NEG, base=256 * k0 - P * j,
                    pattern=([[256, 2], [1, P]] if mw == 256 else [[1, P]]),
                    channel_multiplier=-1,
                )
                nc.scalar.activation(
                    pt[:, 0:mw], dg[:, :mw],
                    mybir.ActivationFunctionType.Exp,
                    bias=0.0, scale=SCALE,
                )
                if qw > mw:
                    nc.scalar.activation(
                        pt[:, mw:qw], st[:, mw:qw],
                        mybir.ActivationFunctionType.Exp,
                        bias=0.0, scale=SCALE,
                    )
            else:
                nc.scalar.activation(
                    pt[:, qoff:qoff + qw], st[:, :qw],
                    mybir.ActivationFunctionType.Exp,
                    bias=0.0, scale=SCALE,
                )
    return pts


def _build_nc():
    nc = bacc.Bacc("TRN2", target_bir_lowering=False)

    # all inputs host-packed to [128, W] with the 6 d-blocks concatenated
    # along columns -> one DMA descriptor per partition (KB-scale elements)
    m_d = nc.dram_tensor("m", [P, ND * D], BF16, kind="ExternalInput")
    xqA_d = nc.dram_tensor("xqA", [P, ND * 512], BF16, kind="ExternalInput")
    xqB_d = nc.dram_tensor("xqB", [P, ND * 512], BF16, kind="ExternalInput")
    # x keys packed chunk-major: col = 3072*c + 512*d + (j%4)*128, c = j//4
    xk_d = nc.dram_tensor("xk", [P, ND * S], BF16, kind="ExternalInput")
    # x keys again in [j, d] row layout + two ones columns, per-tile blocks
    x2_d = nc.dram_tensor("x2", [P, NK * (D + 2)], BF16, kind="ExternalInput")
    wvT_d = nc.dram_tensor("wvT", [P, ND * D], BF16, kind="ExternalInput")
    fb_d = nc.dram_tensor("fbias", [P, 1], F32, kind="ExternalInput")
    out_d = nc.dram_tensor("out", [H, D], F32, kind="ExternalOutput")

    with tile.TileContext(nc) as tc:
        with (
            tc.tile_pool(name="cst", bufs=1) as cst,
            tc.tile_pool(name="xp", bufs=1) as xp,
            tc.tile_pool(name="x2p", bufs=1) as x2p,
            tc.tile_pool(name="wvp", bufs=1) as wvp,
            tc.tile_pool(name="ttp", bufs=ND) as ttp,
            tc.tile_pool(name="ptp", bufs=NK) as ptp,
            tc.tile_pool(name="dgp", bufs=2) as dgp,
            tc.tile_pool(name="sgp", bufs=2) as sgp,
            tc.tile_pool(name="pxp", bufs=3) as pxp,
            tc.tile_pool(name="pxtp", bufs=NQT) as pxtp,
            tc.tile_pool(name="op", bufs=2) as op,
        ):
            fb = cst.tile([P, 1], F32)
            nc.sync.dma_start(out=fb[:], in_=fb_d[:, :])

            tts = []
            xk = xp.tile([P, ND * S], BF16)
            x2 = x2p.tile([P, NK * (D + 2)], BF16)
            wvsb = wvp.tile([P, ND * D], BF16)
            # ---- TT projection (m/xq pools scoped so their SBUF+PSUM free) ----
            with (
                tc.tile_pool(name="mp", bufs=1) as mp,
                tc.tile_pool(name="xqp", bufs=2) as xqp,
                tc.tile_pool(name="psj", bufs=4, space="PSUM") as psj,
            ):
                # two HW queues in parallel for the TT critical path:
                # scalar queue streams m in et-chunks (group et needs only
                # chunk et) then xqB, x2, wv; sync queue delivers xqA then
                # the key chunks in scores-consumption order (c3 first).
                msb = mp.tile([P, ND * D], BF16)
                for et in range(ND):
                    nc.scalar.dma_start(
                        out=msb[:, D * et:D * (et + 1)],
                        in_=m_d[:, D * et:D * (et + 1)])
                xqB = xqp.tile([P, ND * 512], BF16)
                nc.scalar.dma_start(out=xqB[:], in_=xqB_d[:, :])
                xqA = xqp.tile([P, ND * 512], BF16)
                nc.sync.dma_start(out=xqA[:], in_=xqA_d[:, :])
                for c in (3, 0, 1, 2):
                    nc.sync.dma_start(
                        out=xk[:, 3072 * c:3072 * (c + 1)],
                        in_=xk_d[:, 3072 * c:3072 * (c + 1)])
                for half in range(2):
                    w = NK * (D + 2) // 2
                    nc.scalar.dma_start(
                        out=x2[:, w * half:w * (half + 1)],
                        in_=x2_d[:, w * half:w * (half + 1)])
                nc.scalar.dma_start(out=wvsb[:], in_=wvT_d[:, :])

                def xcol(d, j):
                    # column of key tile j's d-block in the chunk-major layout
                    return 3072 * (j // 4) + 512 * d + 128 * (j % 4)

                # TT[d', i] = sum_d M[d, d'] xq^T[d, i]  (qc outer: the first
                # six groups only need m + xqA)
                for et in range(ND):
                    tt = ttp.tile([P, H], BF16)
                    tts.append(tt)
                for qi, xq in enumerate((xqA, xqB)):
                    for et in range(ND):
                        acc = psj.tile([P, 512], F32, tag="ps")
                        for d in range(ND):
                            nc.tensor.matmul(
                                acc[:],
                                msb[:, D * et + P * d:D * et + P * (d + 1)],
                                xq[:, 512 * d:512 * (d + 1)],
                                start=(d == 0), stop=(d == ND - 1),
                            )
                        nc.vector.tensor_copy(tts[et][:, 512 * qi:512 * (qi + 1)], acc[:])

            # ---- scores + exp -> PT tiles (own PSUM scope) ----
            with tc.tile_pool(name="pss", bufs=3, space="PSUM") as pss:
                pts = _scores_phase(nc, tc, fb, xk, xcol, tts, ptp, dgp, pss)

            # ---- Px = P @ [x | 1 1] per query tile k, then out = Pxn @ wv^T.
            # The ones columns give the softmax denominator in px[:, 768];
            # the divide lands on the bf16 Pxn copy; the d<->i transpose for
            # the final contraction runs on the DMA xbar, not the PE. fin(k)
            # is scheduled after px(k-1) so the divide+transpose latency of
            # px(k) hides under the px(k-1) matmul chain.
            with (
                tc.tile_pool(name="ppx", bufs=3, space="PSUM") as ppx,
                tc.tile_pool(name="pfin", bufs=1, space="PSUM") as pfin,
            ):
                def px_chain(k):
                    px = ppx.tile([P, D + 2], F32, tag="px")
                    js = [15] + list(range(2 * k + 1))
                    for idx, j in enumerate(js):
                        k0 = 0 if j == 15 else _k0(j)
                        koff = (k - k0) * P
                        for e0, ew in ((0, 512), (512, D + 2 - 512)):
                            nc.tensor.matmul(
                                px[:, e0:e0 + ew],
                                pts[j][:, koff:koff + P],
                                x2[:, (D + 2) * j + e0:(D + 2) * j + e0 + ew],
                                start=(idx == 0), stop=(idx == len(js) - 1),
                            )
                    rcp = sgp.tile([P, 1], F32, tag="rcp")
                    nc.vector.reciprocal(rcp[:], px[:, D:D + 1])
                    pxn = pxp.tile([P, D], BF16, tag="pxn")
                    nc.vector.tensor_scalar_mul(pxn[:], px[:, :D], rcp[:])
                    pxt = pxtp.tile([P, ND, P], BF16, tag="pxt")
                    nc.sync.dma_start_transpose(out=pxt[:], in_=pxn[:])
                    return pxt

                def fin_chain(k, pxt):
                    fin = pfin.tile([P, D], F32, tag="fin")
                    for di in range(ND):
                        for e0, ew in ((0, 512), (512, 256)):
                            nc.tensor.matmul(
                                fin[:, e0:e0 + ew],
                                pxt[:, di, :],
                                wvsb[:, D * di + e0:D * di + e0 + ew],
                                start=(di == 0), stop=(di == ND - 1),
                            )
                    # drain the single fin PSUM buffer fast: Vector and
                    # Scalar each copy half, halving the next chain's wait
                    o = op.tile([P, D], F32, tag="o")
                    nc.vector.tensor_copy(o[:, 0:384], fin[:, 0:384])
                    nc.scalar.activation(
                        o[:, 384:D], fin[:, 384:D],
                        mybir.ActivationFunctionType.Copy)
                    nc.sync.dma_start(out=out_d[k * P:(k + 1) * P, :], in_=o[:])

                # all px chains first (divides + xbar transposes trail on
                # Vector/DMA), then all fin chains - by fin time every pxt
                # is ready, so the PE never waits on the transpose latency
                pxts = {k: px_chain(k) for k in range(NQT - 1, -1, -1)}
                for k in range(NQT - 1, -1, -1):
                    fin_chain(k, pxts.pop(k))

    nc.compile()
    return nc


def _get_nc():
    if "nc" not in _cached:
        _cached["nc"] = _build_nc()
    return _cached["nc"]


def kernel(x, w_q, w_k, w_v):
    global last_results
    x = np.ascontiguousarray(np.asarray(x, dtype=np.float32))
    w_q = np.asarray(w_q, dtype=np.float32)
    w_k = np.asarray(w_k, dtype=np.float32)
    w_v = np.asarray(w_v, dtype=np.float32)

    def pack_w(w):
        # [768, 768] -> [128, 6*768] with d-blocks along columns
        return np.ascontiguousarray(
            w.reshape(ND, P, D).transpose(1, 0, 2).reshape(P, ND * D)).astype(BF)

    # m packed et-major: col = 768*et + 128*d + c  ->  M[128d+p, 128et+c]
    m = np.ascontiguousarray(
        (w_q.T @ w_k).reshape(ND, P, ND, P).transpose(1, 2, 0, 3).reshape(P, ND * D)
    ).astype(BF)
    wvT = pack_w(np.ascontiguousarray(w_v.T))

    nc = _get_nc()
    in_maps = []
    for core in range(8):
        b, h = core // 2, core % 2
        r = P * h
        rot = np.concatenate([x[b, r:], x[b, :r]], axis=0)
        xT = np.ascontiguousarray(rot.T)                      # [768, 2048]
        xk = np.ascontiguousarray(
            xT.reshape(ND, P, 4, 512).transpose(1, 2, 0, 3).reshape(P, ND * S)
        ).astype(BF)
        xqT = np.ascontiguousarray(
            x[b].reshape(NK, P, D)[h::2].reshape(H, D).T)     # [768, 1024]
        xqA = np.ascontiguousarray(
            xqT[:, 0:512].reshape(ND, P, 512).transpose(1, 0, 2).reshape(P, ND * 512)
        ).astype(BF)
        xqB = np.ascontiguousarray(
            xqT[:, 512:H].reshape(ND, P, 512).transpose(1, 0, 2).reshape(P, ND * 512)
        ).astype(BF)
        x2 = np.ascontiguousarray(
            np.concatenate([rot, np.ones((S, 2), np.float32)], axis=1)
            .reshape(NK, P, D + 2).transpose(1, 0, 2).reshape(P, NK * (D + 2))
        ).astype(BF)
        in_maps.append({
            "m": m,
            "xqA": xqA,
            "xqB": xqB,
            "xk": xk,
            "x2": x2,
            "wvT": wvT,
            "fbias": np.full((P, 1), 0.0 if h == 1 else NEG, np.float32),
        })

    trace = bool(int(os.environ.get("KERNEL_TRACE", "0")))
    res = run_bass_kernel_spmd(nc, in_maps, core_ids=list(range(8)), trace=trace)
    last_results = res

    out = np.empty((B, S, D), np.float32)
    for core in range(8):
        b, h = core // 2, core % 2
        o = res.results[core]["out"]
        out[b].reshape(NK, P, D)[h::2] = o.reshape(NQT, P, D)
    return out
